# revision 14
# baseline (speedup 1.0000x reference)
# Multi-head attention (B=2, T=4096, DIM=1024, H=16, D=64) with RoPE,
# tensor-parallel over 8 TRN2 NeuronCores: core c handles batch c//4 and
# heads 4*(c%4) .. 4*(c%4)+3. Each core computes its 4 heads end-to-end and
# a partial output projection (row-parallel wo); the host sums the 4
# partials per batch and transposes back.
#
# v2 structure:
#  - Phase A fuses the projections with the (hp=0, tw=0) attention pass:
#    as soon as a kT/v s-block is projected+transposed, its QK/exp/PV for
#    the first query window runs, so the scalar engine starts exp work
#    ~100us earlier than a strict phase split.
#  - Out-projection contracts K=128 (two heads' d-rows stacked on
#    partitions, yP/woS layout) instead of 4 separate K=65 matmuls.
#  - exp is split across engines: most tiles on ACT (table exp), a tunable
#    subset on DVE/GPSIMD via a bf16 Schraudolph bit-trick (tensor_scalar
#    mult+add -> int16 -> bitcast bf16), trading ~2-3% relative error on
#    those s-blocks for scalar-engine headroom.
import numpy as np
import ml_dtypes

B, T, DIM = 2, 4096, 1024
HEADS, HD = 16, 64
N_CORES = 8
HPC = 4          # heads per core
JC = HPC * HD    # 256 projection cols per core
BF16 = ml_dtypes.bfloat16

# exp engine routing in phase B, keyed by sb % 8
DVE_SB = (1, 3, 5, 7)  # -> DVE fast-exp
GPS_SB = ()       # gpsimd tensor_scalar fails to lower on HW; unused
# fast-exp: bf16 bits = (s * FE_A) + FE_B, computed fp32, converted int16
FE_A = 0.125 * 128.0 * 1.4426950408889634
FE_B = 128.0 * 127.0 - 5.51

_PROGRAM = None  # cached program


def _rope_tables_np():
    inv_freq = 1.0 / (10000.0 ** (np.arange(0, HD, 2, dtype=np.float32) / HD))
    t = np.arange(T, dtype=np.float32)
    freqs = np.einsum("i,j->ij", t, inv_freq).astype(np.float32)  # [T, 32]
    emb = np.concatenate((freqs, freqs), axis=-1)  # [T, 64]
    cos = np.cos(emb).astype(np.float32)
    sin = np.sin(emb).astype(np.float32)
    sgn = np.where(np.arange(HD) < 32, -1.0, 1.0).astype(np.float32)
    sinS = sin * sgn[None, :]
    return cos, sinS


def _build_program():
    from concourse import bacc
    import concourse.mybir as mybir
    import concourse.tile as tile
    from concourse.masks import make_identity

    BF = mybir.dt.bfloat16
    F32 = mybir.dt.float32
    I16 = mybir.dt.int16
    AF = mybir.ActivationFunctionType
    MUL = mybir.AluOpType.mult
    ADD = mybir.AluOpType.add

    nc = bacc.Bacc("TRN2", debug=False, num_devices=N_CORES)

    xT = nc.dram_tensor("xT", [DIM, T], BF, kind="ExternalInput")
    wqT = nc.dram_tensor("wqT", [DIM, JC], BF, kind="ExternalInput")
    wkT = nc.dram_tensor("wkT", [DIM, JC], BF, kind="ExternalInput")
    wvT = nc.dram_tensor("wvT", [DIM, JC], BF, kind="ExternalInput")
    woS = nc.dram_tensor("woS", [128, 2, DIM], BF, kind="ExternalInput")
    cosn = nc.dram_tensor("cosn", [T, HD], F32, kind="ExternalInput")
    sinn = nc.dram_tensor("sinn", [T, HD], F32, kind="ExternalInput")
    chain = nc.dram_tensor("chain", [DIM, T], F32, kind="ExternalInput")
    pout = nc.dram_tensor("pout", [DIM, T], F32, kind="ExternalOutput")
    chk = nc.dram_tensor("chk", [1, 512], F32, kind="ExternalOutput")

    NCC = DIM // 128     # 8 contraction chunks
    NTB = T // 128       # 32 t/s-blocks of 128
    NSB = NTB
    NTW = T // 512       # 8 query windows of 512

    with tile.TileContext(nc) as tc:
        with (
            tc.tile_pool(name="const", bufs=1) as constp,
            tc.tile_pool(name="xp", bufs=4) as xp,
            tc.tile_pool(name="ropep", bufs=6) as ropep,
            tc.tile_pool(name="ptp", bufs=10) as ptp,
            tc.tile_pool(name="ptd", bufs=4) as ptd,
            tc.tile_pool(name="addp", bufs=2) as addp,
            tc.tile_pool(name="stagep", bufs=3) as stagep,
            tc.tile_pool(name="normp", bufs=2) as normp,
        ):
            # ---- persistent tiles ----
            ident = constp.tile([128, 128], BF)
            make_identity(nc, ident)

            # fused q|k weights: cols 0:256 = wq, 256:512 = wk
            wqk_s = constp.tile([128, NCC, 2 * JC], BF)
            nc.sync.dma_start(
                wqk_s[:, :, 0:JC], wqT.ap().rearrange("(cc p) j -> p cc j", p=128)
            )
            nc.sync.dma_start(
                wqk_s[:, :, JC : 2 * JC],
                wkT.ap().rearrange("(cc p) j -> p cc j", p=128),
            )
            wv_s = constp.tile([128, NCC, JC], BF)
            wo_s = constp.tile([128, 2, DIM], BF)
            cos_s = constp.tile([128, NTB, HD], F32)
            sin_s = constp.tile([128, NTB, HD], F32)

            zbias = constp.tile([128, 1], F32)
            nc.vector.memset(zbias, 0.0)

            chtile = constp.tile([1, 512], F32)
            nc.sync.dma_start(chtile, chain.ap()[0:1, 0:512])
            nc.sync.dma_start(chk.ap(), chtile)

            qTs = constp.tile([128, 2, T], BF)   # [j, t]; j=jb*128+p, head=j//64
            kTs = constp.tile([128, 2, T], BF)
            # v in normal layout per s-chunk; per head: col 0..63 = d, 64 = ones
            # (ones last so PV output rows 0:64 = head dims at base partition 0
            # and row 64 = softmax denominator — partition-aligned reads)
            v_s = constp.tile([128, NSB, HPC * (HD + 1)], BF)
            # normalized attention out, pair-stacked: partitions 0:64 head 2p,
            # 64:128 head 2p+1 (junk denominator row dropped at normalize)
            yP = constp.tile([128, 2, T], BF)

            v4 = v_s.rearrange("p sc (h u) -> p sc h u", h=HPC)
            for h in range(HPC):
                nc.vector.memset(v4[:, :, h, HD : HD + 1], 1.0)

            # ---------- helpers ----------
            def emit_qk(hp, tw, sb, pool):
                twsl = slice(tw * 512, (tw + 1) * 512)
                ssl = slice(sb * 128, (sb + 1) * 128)
                sAB = pool.tile([128, 1024], F32, tag="s")
                nc.tensor.matmul(
                    sAB[:, 0:512],
                    lhsT=kTs[0:64, hp, ssl], rhs=qTs[0:64, hp, twsl],
                    start=True, stop=True,
                )
                nc.tensor.matmul(
                    sAB[:, 512:1024],
                    lhsT=kTs[64:128, hp, ssl], rhs=qTs[64:128, hp, twsl],
                    start=True, stop=True,
                )
                return sAB

            def emit_exp(sAB, route):
                if route == 0:
                    pAB = ptp.tile([128, 1024], BF, tag="pT")
                    nc.scalar.activation(pAB, sAB, AF.Exp, bias=zbias, scale=0.125)
                    return pAB
                elif route == 1:
                    t16 = ptd.tile([128, 1024], I16, tag="p16")
                    nc.vector.tensor_scalar(t16, sAB, FE_A, FE_B, MUL, ADD)
                    return t16.bitcast(BF)
                else:
                    raise ValueError("gpsimd exp route unavailable")

            def emit_pv(hp, sb, pAB, oA, oB):
                hA, hB = 2 * hp, 2 * hp + 1
                nc.tensor.matmul(
                    oA[0 : HD + 1, :],
                    lhsT=v_s[:, sb, hA * 65 : hA * 65 + 65],
                    rhs=pAB[:, 0:512],
                    start=(sb == 0), stop=(sb == NSB - 1),
                )
                nc.tensor.matmul(
                    oB[0 : HD + 1, :],
                    lhsT=v_s[:, sb, hB * 65 : hB * 65 + 65],
                    rhs=pAB[:, 512:1024],
                    start=(sb == 0), stop=(sb == NSB - 1),
                )

            def norm_pair(hp, tw, oA, oB):
                # yP[0:64, hp, tw] = oA[0:64]/oA[64]; yP[64:128,...] = oB
                twsl = slice(tw * 512, (tw + 1) * 512)
                for half, o in ((0, oA), (1, oB)):
                    rc = normp.tile([1, 512], F32, tag="rc")
                    nc.vector.reciprocal(rc, o[HD : HD + 1, :])
                    bc = normp.tile([64, 512], F32, tag="bc")
                    nc.gpsimd.partition_broadcast(bc, rc)
                    nc.vector.tensor_mul(
                        yP[64 * half : 64 * half + 64, hp, twsl],
                        o[0:HD, :], bc,
                    )

            # ---- phase A: projections + RoPE + transpose, fused with the
            # (hp=0, tw=0) attention stream ----
            with (
                tc.tile_pool(name="psSa", bufs=1, space="PSUM") as psSa,
                tc.tile_pool(name="psA", bufs=2, space="PSUM") as psA,
                tc.tile_pool(name="psP0", bufs=1, space="PSUM") as psP0,
                tc.tile_pool(name="psP1", bufs=1, space="PSUM") as psP1,
                tc.tile_pool(name="psT", bufs=2, space="PSUM") as psT,
            ):
                oA0 = psA.tile([128, 512], F32, tag="o")
                oB0 = psA.tile([128, 512], F32, tag="o")

                trans_pending = []

                def emit_transposes(keep=0):
                    while len(trans_pending) > keep:
                        qr_, dstT_, tsl_ = trans_pending.pop(0)
                        for jb in range(2):
                            tp = psT.tile([128, 128], BF, tag="tp")
                            nc.tensor.transpose(
                                tp, qr_[:, jb * 128 : (jb + 1) * 128], ident
                            )
                            nc.vector.tensor_copy(dstT_[:, jb, tsl_], tp)

                def rope_of(P, tb, dstT):
                    # P: [128, JC] fp32 view (psum); RoPE -> bf16 -> pend
                    tsl = slice(tb * 128, (tb + 1) * 128)
                    A = ropep.tile([128, JC], F32, tag="A")
                    P4 = P.rearrange("p (h d) -> p h d", h=HPC)
                    ct = (
                        cos_s[:, tb, :]
                        .rearrange("p (o d) -> p o d", o=1)
                        .broadcast_to([128, HPC, HD])
                    )
                    nc.vector.tensor_mul(
                        A.rearrange("p (h d) -> p h d", h=HPC), P4, ct
                    )
                    Bt = ropep.tile([128, JC], F32, tag="B")
                    B4 = Bt.rearrange("p (h u d) -> p h u d", h=HPC, u=2)
                    P42 = P.rearrange("p (h u d) -> p h u d", h=HPC, u=2)
                    s0 = (
                        sin_s[:, tb, 0:32]
                        .rearrange("p (o d) -> p o d", o=1)
                        .broadcast_to([128, HPC, 32])
                    )
                    s1 = (
                        sin_s[:, tb, 32:64]
                        .rearrange("p (o d) -> p o d", o=1)
                        .broadcast_to([128, HPC, 32])
                    )
                    nc.vector.tensor_mul(B4[:, :, 0, :], P42[:, :, 1, :], s0)
                    nc.vector.tensor_mul(B4[:, :, 1, :], P42[:, :, 0, :], s1)
                    qr = ropep.tile([128, JC], BF, tag="qr")
                    nc.vector.tensor_add(qr, A, Bt)
                    trans_pending.append((qr, dstT, tsl))

                def proj_qk(tb, xt):
                    # fused q|k projection, K=64 row-split into two
                    # independent accumulators (row groups alternate, so
                    # LDWEIGHTS pull-ahead keeps PE streaming); halves are
                    # summed on DVE before RoPE
                    P0 = psP0.tile([128, 2 * JC], F32, tag="P0")
                    P1 = psP1.tile([128, 2 * JC], F32, tag="P1")
                    for cc in range(NCC):
                        nc.tensor.matmul(
                            P0, lhsT=xt[0:64, cc, :], rhs=wqk_s[0:64, cc, :],
                            start=(cc == 0), stop=(cc == NCC - 1),
                        )
                        nc.tensor.matmul(
                            P1, lhsT=xt[64:128, cc, :], rhs=wqk_s[64:128, cc, :],
                            start=(cc == 0), stop=(cc == NCC - 1),
                        )
                    # tensor ops can't read two PSUM operands; stage P1
                    # through SBUF on the (phase-A-idle) scalar engine
                    P1c = addp.tile([128, 2 * JC], F32, tag="P1c")
                    nc.scalar.copy(P1c, P1)
                    Ps = addp.tile([128, 2 * JC], F32, tag="Ps")
                    nc.vector.tensor_add(Ps, P0, P1c)
                    rope_of(Ps[:, 0:JC], tb, qTs)
                    rope_of(Ps[:, JC : 2 * JC], tb, kTs)

                def vproj(tb, xt):
                    # row-split like q|k, reusing the q|k PSUM slots (the
                    # WAR dep on the q|k add is hidden behind the attention
                    # stream emitted before this)
                    V0 = psP0.tile([128, 2 * JC], F32, tag="P0")
                    V1 = psP1.tile([128, 2 * JC], F32, tag="P1")
                    for cc in range(NCC):
                        nc.tensor.matmul(
                            V0[:, 0:JC], lhsT=xt[0:64, cc, :],
                            rhs=wv_s[0:64, cc, :],
                            start=(cc == 0), stop=(cc == NCC - 1),
                        )
                        nc.tensor.matmul(
                            V1[:, 0:JC], lhsT=xt[64:128, cc, :],
                            rhs=wv_s[64:128, cc, :],
                            start=(cc == 0), stop=(cc == NCC - 1),
                        )
                    V1c = addp.tile([128, JC], F32, tag="V1c")
                    nc.scalar.copy(V1c, V1[:, 0:JC])
                    nc.vector.tensor_tensor(
                        v4[:, tb, :, 0:HD],
                        V0[:, 0:JC].rearrange("p (h d) -> p h d", h=HPC),
                        V1c.rearrange("p (h d) -> p h d", h=HPC),
                        mybir.AluOpType.add,
                    )

                def attn_sb_a(sb):
                    # phase-A attention (hp=0, tw=0), one head at a time so
                    # the scores fit a single PSUM bank
                    ssl = slice(sb * 128, (sb + 1) * 128)
                    for half, o in ((0, oA0), (1, oB0)):
                        hsl = slice(64 * half, 64 * half + 64)
                        sS = psSa.tile([128, 512], F32, tag="s")
                        nc.tensor.matmul(
                            sS, lhsT=kTs[hsl, 0, ssl], rhs=qTs[hsl, 0, 0:512],
                            start=True, stop=True,
                        )
                        pS = ptp.tile([128, 512], BF, tag="pA")
                        nc.scalar.activation(pS, sS, AF.Exp, bias=zbias, scale=0.125)
                        nc.tensor.matmul(
                            o[0 : HD + 1, :],
                            lhsT=v_s[:, sb, half * 65 : half * 65 + 65],
                            rhs=pS,
                            start=(sb == 0), stop=(sb == NSB - 1),
                        )

                done_sb = 0
                for tb in range(NTB):
                    xt = xp.tile([128, NCC, 128], BF, tag="xt")
                    nc.sync.dma_start(
                        xt,
                        xT.ap().rearrange("(cc p) t -> p cc t", p=128)[
                            :, :, tb * 128 : (tb + 1) * 128
                        ],
                    )
                    if tb == 0:
                        nc.sync.dma_start(
                            wv_s, wvT.ap().rearrange("(cc p) j -> p cc j", p=128)
                        )
                        nc.sync.dma_start(
                            cos_s, cosn.ap().rearrange("(tc p) d -> p tc d", p=128)
                        )
                        nc.sync.dma_start(
                            sin_s, sinn.ap().rearrange("(tc p) d -> p tc d", p=128)
                        )
                        nc.sync.dma_start(wo_s, woS.ap())
                    proj_qk(tb, xt)
                    emit_transposes(keep=2)
                    # stream the (hp0, tw0) attention for ready s-blocks
                    # (tb-5: tw0's q transposes land by tb=5 with the keep=2 lag)
                    while done_sb <= tb - 5:
                        attn_sb_a(done_sb)
                        done_sb += 1
                    vproj(tb, xt)
                emit_transposes()
                while done_sb < NSB:
                    attn_sb_a(done_sb)
                    done_sb += 1
                norm_pair(0, 0, oA0, oB0)

            # ---- phase B: remaining 15 (hp, tw) attention pairs with
            # engine-routed exp; out-projection dripped ----
            with (
                tc.tile_pool(name="psS", bufs=2, space="PSUM") as psS,
                tc.tile_pool(name="psO", bufs=4, space="PSUM") as psO,
            ):
                GRP = 8
                outproj_pending = []

                def emit_outproj(n):
                    while outproj_pending:
                        if n <= 0:
                            return
                        n -= 1
                        tw_, cb = outproj_pending.pop(0)
                        cbsl = slice(cb * 128, (cb + 1) * 128)
                        osl = slice(tw_ * 512, (tw_ + 1) * 512)
                        po = psO.tile([128, 512], F32, tag="o")
                        for pair in range(2):
                            nc.tensor.matmul(
                                po, lhsT=wo_s[:, pair, cbsl], rhs=yP[:, pair, osl],
                                start=(pair == 0), stop=(pair == 1),
                            )
                        st = stagep.tile([128, 512], F32, tag="st")
                        nc.vector.tensor_copy(st, po)
                        nc.sync.dma_start(pout.ap()[cbsl, osl], st)

                pairs = [(1, 0)] + [
                    (hp, tw) for tw in range(1, NTW) for hp in range(2)
                ]
                for hp, tw in pairs:
                    oA = psO.tile([128, 512], F32, tag="o")
                    oB = psO.tile([128, 512], F32, tag="o")
                    for g in range(NSB // GRP):
                        pend = []
                        for i in range(GRP):
                            sb = g * GRP + i
                            sAB = emit_qk(hp, tw, sb, psS)
                            m = sb % 8
                            route = 1 if m in DVE_SB else (2 if m in GPS_SB else 0)
                            pend.append((sb, emit_exp(sAB, route)))
                        emit_outproj(2)
                        for sb, pAB in pend:
                            emit_pv(hp, sb, pAB, oA, oB)
                    norm_pair(hp, tw, oA, oB)
                    if hp == 1:
                        emit_outproj(8)  # drain leftovers
                        outproj_pending.extend((tw, cb) for cb in range(8))
                emit_outproj(8)  # final tw's out-projection

    nc.compile()
    return nc


def _get_program():
    global _PROGRAM
    if _PROGRAM is None:
        _PROGRAM = _build_program()
    return _PROGRAM


def make_in_maps(x, wq, wk, wv, wo):
    """Host-side sharding/layout prep: per-core input dicts."""
    x = np.asarray(x, dtype=np.float32)
    wq = np.asarray(wq, dtype=np.float32)
    wk = np.asarray(wk, dtype=np.float32)
    wv = np.asarray(wv, dtype=np.float32)
    wo = np.asarray(wo, dtype=np.float32)
    cos, sinS = _rope_tables_np()

    xT_b = [np.ascontiguousarray(x[b].T).astype(BF16) for b in range(B)]
    in_maps = []
    for c in range(N_CORES):
        b, hg = divmod(c, HPC)
        jsl = slice(hg * JC, (hg + 1) * JC)
        wqTc = np.ascontiguousarray(wq[jsl, :].T).astype(BF16)
        wkTc = np.ascontiguousarray(wk[jsl, :].T).astype(BF16)
        wvTc = np.ascontiguousarray(wv[jsl, :].T).astype(BF16)
        # woS[p, pair, co]: p<64 -> head 2*pair d=p ; p>=64 -> head 2*pair+1
        wo_cols = wo[:, jsl].reshape(DIM, HPC, HD)  # [co, h, d]
        woSc = np.zeros((128, 2, DIM), dtype=np.float32)
        for pair in range(2):
            woSc[0:64, pair, :] = wo_cols[:, 2 * pair, :].T
            woSc[64:128, pair, :] = wo_cols[:, 2 * pair + 1, :].T
        in_maps.append(
            {
                "xT": xT_b[b],
                "wqT": wqTc,
                "wkT": wkTc,
                "wvT": wvTc,
                "woS": woSc.astype(BF16),
                "cosn": cos,
                "sinn": sinS,
                "chain": _ZCHAIN,
            }
        )
    return in_maps


_ZCHAIN = np.zeros((DIM, T), dtype=np.float32)


def assemble(results):
    """Host-side unshard: sum 4 head-group partials per batch, transpose."""
    out = np.zeros((B, T, DIM), dtype=np.float32)
    for b in range(B):
        acc = np.zeros((DIM, T), dtype=np.float32)
        for hg in range(HPC):
            acc += results[b * HPC + hg]["pout"]
        out[b] = acc.T
    return out


def kernel(x, wq, wk, wv, wo):
    from concourse.bass_utils import run_bass_kernel_spmd

    nc = _get_program()
    in_maps = make_in_maps(x, wq, wk, wv, wo)
    res = run_bass_kernel_spmd(nc, in_maps, core_ids=list(range(N_CORES)))
    return assemble(res.results)


if __name__ == "__main__":
    nc = _get_program()
    print("program built + compiled OK")


# revision 15
# speedup vs baseline: 1.3851x; 1.3851x over previous
# Multi-head attention (B=2, T=4096, DIM=1024, H=16, D=64) with RoPE,
# tensor-parallel over 8 TRN2 NeuronCores: core c handles batch c//4 and
# heads 4*(c%4) .. 4*(c%4)+3. Each core computes its 4 heads end-to-end and
# a partial output projection (row-parallel wo); the host sums the 4
# partials per batch and transposes back.
#
# v2 structure:
#  - Phase A fuses the projections with the (hp=0, tw=0) attention pass:
#    as soon as a kT/v s-block is projected+transposed, its QK/exp/PV for
#    the first query window runs, so the scalar engine starts exp work
#    ~100us earlier than a strict phase split.
#  - Out-projection contracts K=128 (two heads' d-rows stacked on
#    partitions, yP/woS layout) instead of 4 separate K=65 matmuls.
#  - exp is split across engines: most tiles on ACT (table exp), a tunable
#    subset on DVE/GPSIMD via a bf16 Schraudolph bit-trick (tensor_scalar
#    mult+add -> int16 -> bitcast bf16), trading ~2-3% relative error on
#    those s-blocks for scalar-engine headroom.
import numpy as np
import ml_dtypes

B, T, DIM = 2, 4096, 1024
HEADS, HD = 16, 64
N_CORES = 8
HPC = 4          # heads per core
JC = HPC * HD    # 256 projection cols per core
BF16 = ml_dtypes.bfloat16

# exp engine routing in phase B, keyed by sb % 8
DVE_SB = (1, 3, 5, 7)  # -> DVE fast-exp
GPS_SB = ()       # gpsimd tensor_scalar fails to lower on HW; unused
# fast-exp: bf16 bits = (s * FE_A) + FE_B, computed fp32, converted int16
FE_A = 0.125 * 128.0 * 1.4426950408889634
FE_B = 128.0 * 127.0 - 5.51

_PROGRAM = None  # cached program


def _rope_tables_np():
    inv_freq = 1.0 / (10000.0 ** (np.arange(0, HD, 2, dtype=np.float32) / HD))
    t = np.arange(T, dtype=np.float32)
    freqs = np.einsum("i,j->ij", t, inv_freq).astype(np.float32)  # [T, 32]
    emb = np.concatenate((freqs, freqs), axis=-1)  # [T, 64]
    cos = np.cos(emb).astype(np.float32)
    sin = np.sin(emb).astype(np.float32)
    sgn = np.where(np.arange(HD) < 32, -1.0, 1.0).astype(np.float32)
    sinS = sin * sgn[None, :]
    return cos, sinS


def _build_program():
    from concourse import bacc
    import concourse.mybir as mybir
    import concourse.tile as tile
    from concourse.masks import make_identity

    BF = mybir.dt.bfloat16
    F32 = mybir.dt.float32
    I16 = mybir.dt.int16
    AF = mybir.ActivationFunctionType
    MUL = mybir.AluOpType.mult
    ADD = mybir.AluOpType.add

    nc = bacc.Bacc("TRN2", debug=False, num_devices=N_CORES)

    xT = nc.dram_tensor("xT", [DIM, T], BF, kind="ExternalInput")
    wqT = nc.dram_tensor("wqT", [DIM, JC], BF, kind="ExternalInput")
    wkT = nc.dram_tensor("wkT", [DIM, JC], BF, kind="ExternalInput")
    wvT = nc.dram_tensor("wvT", [DIM, JC], BF, kind="ExternalInput")
    woS = nc.dram_tensor("woS", [128, 2, DIM], BF, kind="ExternalInput")
    cosn = nc.dram_tensor("cosn", [T, HD], F32, kind="ExternalInput")
    sinn = nc.dram_tensor("sinn", [T, HD], F32, kind="ExternalInput")
    chain = nc.dram_tensor("chain", [DIM, T], F32, kind="ExternalInput")
    pout = nc.dram_tensor("pout", [DIM, T], F32, kind="ExternalOutput")
    chk = nc.dram_tensor("chk", [1, 512], F32, kind="ExternalOutput")

    NCC = DIM // 128     # 8 contraction chunks
    NTB = T // 128       # 32 t/s-blocks of 128
    NSB = NTB
    NTW = T // 512       # 8 query windows of 512

    with tile.TileContext(nc) as tc:
        with (
            tc.tile_pool(name="const", bufs=1) as constp,
            tc.tile_pool(name="xp", bufs=4) as xp,
            tc.tile_pool(name="ropep", bufs=6) as ropep,
            tc.tile_pool(name="ptp", bufs=10) as ptp,
            tc.tile_pool(name="ptd", bufs=4) as ptd,
            tc.tile_pool(name="addp", bufs=2) as addp,
            tc.tile_pool(name="stagep", bufs=3) as stagep,
            tc.tile_pool(name="normp", bufs=2) as normp,
        ):
            # ---- persistent tiles ----
            ident = constp.tile([128, 128], BF)
            make_identity(nc, ident)

            # fused q|k weights: cols 0:256 = wq, 256:512 = wk
            wqk_s = constp.tile([128, NCC, 2 * JC], BF)
            nc.sync.dma_start(
                wqk_s[:, :, 0:JC], wqT.ap().rearrange("(cc p) j -> p cc j", p=128)
            )
            nc.sync.dma_start(
                wqk_s[:, :, JC : 2 * JC],
                wkT.ap().rearrange("(cc p) j -> p cc j", p=128),
            )
            wv_s = constp.tile([128, NCC, JC], BF)
            wo_s = constp.tile([128, 2, DIM], BF)
            cos_s = constp.tile([128, NTB, HD], F32)
            sin_s = constp.tile([128, NTB, HD], F32)

            zbias = constp.tile([128, 1], F32)
            nc.vector.memset(zbias, 0.0)

            chtile = constp.tile([1, 512], F32)
            nc.sync.dma_start(chtile, chain.ap()[0:1, 0:512])
            nc.sync.dma_start(chk.ap(), chtile)

            qTs = constp.tile([128, 2, T], BF)   # [j, t]; j=jb*128+p, head=j//64
            kTs = constp.tile([128, 2, T], BF)
            # v in normal layout per s-chunk; per head: col 0..63 = d, 64 = ones
            # (ones last so PV output rows 0:64 = head dims at base partition 0
            # and row 64 = softmax denominator — partition-aligned reads)
            v_s = constp.tile([128, NSB, HPC * (HD + 1)], BF)
            # normalized attention out, pair-stacked: partitions 0:64 head 2p,
            # 64:128 head 2p+1 (junk denominator row dropped at normalize)
            yP = constp.tile([128, 2, T], BF)

            v4 = v_s.rearrange("p sc (h u) -> p sc h u", h=HPC)
            for h in range(HPC):
                nc.vector.memset(v4[:, :, h, HD : HD + 1], 1.0)

            # ---------- helpers ----------
            def emit_qk(hp, tw, sb, pool):
                twsl = slice(tw * 512, (tw + 1) * 512)
                ssl = slice(sb * 128, (sb + 1) * 128)
                sAB = pool.tile([128, 1024], F32, tag="s")
                nc.tensor.matmul(
                    sAB[:, 0:512],
                    lhsT=kTs[0:64, hp, ssl], rhs=qTs[0:64, hp, twsl],
                    start=True, stop=True,
                )
                nc.tensor.matmul(
                    sAB[:, 512:1024],
                    lhsT=kTs[64:128, hp, ssl], rhs=qTs[64:128, hp, twsl],
                    start=True, stop=True,
                )
                return sAB

            def emit_exp(sAB, route):
                if route == 0:
                    pAB = ptp.tile([128, 1024], BF, tag="pT")
                    nc.scalar.activation(pAB, sAB, AF.Exp, bias=zbias, scale=0.125)
                    return pAB
                elif route == 1:
                    t16 = ptd.tile([128, 1024], I16, tag="p16")
                    nc.vector.tensor_scalar(t16, sAB, FE_A, FE_B, MUL, ADD)
                    return t16.bitcast(BF)
                else:
                    raise ValueError("gpsimd exp route unavailable")

            def emit_pv(hp, sb, pAB, oA, oB):
                hA, hB = 2 * hp, 2 * hp + 1
                nc.tensor.matmul(
                    oA[0 : HD + 1, :],
                    lhsT=v_s[:, sb, hA * 65 : hA * 65 + 65],
                    rhs=pAB[:, 0:512],
                    start=(sb == 0), stop=(sb == NSB - 1),
                )
                nc.tensor.matmul(
                    oB[0 : HD + 1, :],
                    lhsT=v_s[:, sb, hB * 65 : hB * 65 + 65],
                    rhs=pAB[:, 512:1024],
                    start=(sb == 0), stop=(sb == NSB - 1),
                )

            def norm_pair(hp, tw, oA, oB):
                # yP[0:64, hp, tw] = oA[0:64]/oA[64]; yP[64:128,...] = oB
                twsl = slice(tw * 512, (tw + 1) * 512)
                for half, o in ((0, oA), (1, oB)):
                    rc = normp.tile([1, 512], F32, tag="rc")
                    nc.vector.reciprocal(rc, o[HD : HD + 1, :])
                    bc = normp.tile([64, 512], F32, tag="bc")
                    nc.gpsimd.partition_broadcast(bc, rc)
                    nc.vector.tensor_mul(
                        yP[64 * half : 64 * half + 64, hp, twsl],
                        o[0:HD, :], bc,
                    )

            # ---- phase A: projections + RoPE + transpose, fused with the
            # (hp=0, tw=0) attention stream ----
            with (
                tc.tile_pool(name="psSa", bufs=1, space="PSUM") as psSa,
                tc.tile_pool(name="psA", bufs=2, space="PSUM") as psA,
                tc.tile_pool(name="psP0", bufs=1, space="PSUM") as psP0,
                tc.tile_pool(name="psP1", bufs=1, space="PSUM") as psP1,
                tc.tile_pool(name="psV", bufs=1, space="PSUM") as psV,
                tc.tile_pool(name="psT", bufs=2, space="PSUM") as psT,
            ):
                oA0 = psA.tile([128, 512], F32, tag="o")
                oB0 = psA.tile([128, 512], F32, tag="o")

                trans_pending = []

                def emit_transposes(keep=0):
                    while len(trans_pending) > keep:
                        qr_, dstT_, tsl_ = trans_pending.pop(0)
                        for jb in range(2):
                            tp = psT.tile([128, 128], BF, tag="tp")
                            nc.tensor.transpose(
                                tp, qr_[:, jb * 128 : (jb + 1) * 128], ident
                            )
                            nc.vector.tensor_copy(dstT_[:, jb, tsl_], tp)

                def rope_of(P, tb, dstT):
                    # P: [128, JC] fp32 view (psum); RoPE -> bf16 -> pend
                    tsl = slice(tb * 128, (tb + 1) * 128)
                    A = ropep.tile([128, JC], F32, tag="A")
                    P4 = P.rearrange("p (h d) -> p h d", h=HPC)
                    ct = (
                        cos_s[:, tb, :]
                        .rearrange("p (o d) -> p o d", o=1)
                        .broadcast_to([128, HPC, HD])
                    )
                    nc.vector.tensor_mul(
                        A.rearrange("p (h d) -> p h d", h=HPC), P4, ct
                    )
                    Bt = ropep.tile([128, JC], F32, tag="B")
                    B4 = Bt.rearrange("p (h u d) -> p h u d", h=HPC, u=2)
                    P42 = P.rearrange("p (h u d) -> p h u d", h=HPC, u=2)
                    s0 = (
                        sin_s[:, tb, 0:32]
                        .rearrange("p (o d) -> p o d", o=1)
                        .broadcast_to([128, HPC, 32])
                    )
                    s1 = (
                        sin_s[:, tb, 32:64]
                        .rearrange("p (o d) -> p o d", o=1)
                        .broadcast_to([128, HPC, 32])
                    )
                    nc.vector.tensor_mul(B4[:, :, 0, :], P42[:, :, 1, :], s0)
                    nc.vector.tensor_mul(B4[:, :, 1, :], P42[:, :, 0, :], s1)
                    qr = ropep.tile([128, JC], BF, tag="qr")
                    nc.vector.tensor_add(qr, A, Bt)
                    trans_pending.append((qr, dstT, tsl))

                def proj_qk(tb, xt):
                    # fused q|k projection, K=64 row-split into two
                    # independent accumulators (row groups alternate, so
                    # LDWEIGHTS pull-ahead keeps PE streaming); halves are
                    # summed on DVE before RoPE
                    P0 = psP0.tile([128, 2 * JC], F32, tag="P0")
                    P1 = psP1.tile([128, 2 * JC], F32, tag="P1")
                    for cc in range(NCC):
                        nc.tensor.matmul(
                            P0, lhsT=xt[0:64, cc, :], rhs=wqk_s[0:64, cc, :],
                            start=(cc == 0), stop=(cc == NCC - 1),
                        )
                        nc.tensor.matmul(
                            P1, lhsT=xt[64:128, cc, :], rhs=wqk_s[64:128, cc, :],
                            start=(cc == 0), stop=(cc == NCC - 1),
                        )
                    # tensor ops can't read two PSUM operands; stage P1
                    # through SBUF on the (phase-A-idle) scalar engine
                    P1c = addp.tile([128, 2 * JC], F32, tag="P1c")
                    nc.scalar.copy(P1c, P1)
                    Ps = addp.tile([128, 2 * JC], F32, tag="Ps")
                    nc.vector.tensor_add(Ps, P0, P1c)
                    rope_of(Ps[:, 0:JC], tb, qTs)
                    rope_of(Ps[:, JC : 2 * JC], tb, kTs)

                def vproj(tb, xt):
                    V = psV.tile([128, JC], F32, tag="V")
                    for cc in range(NCC):
                        nc.tensor.matmul(
                            V, lhsT=xt[:, cc, :], rhs=wv_s[:, cc, :],
                            start=(cc == 0), stop=(cc == NCC - 1),
                        )
                    nc.vector.tensor_copy(
                        v4[:, tb, :, 0:HD],
                        V.rearrange("p (h d) -> p h d", h=HPC),
                    )

                def attn_sb_a(sb):
                    # phase-A attention (hp=0, tw=0), one head at a time so
                    # the scores fit a single PSUM bank
                    ssl = slice(sb * 128, (sb + 1) * 128)
                    for half, o in ((0, oA0), (1, oB0)):
                        hsl = slice(64 * half, 64 * half + 64)
                        sS = psSa.tile([128, 512], F32, tag="s")
                        nc.tensor.matmul(
                            sS, lhsT=kTs[hsl, 0, ssl], rhs=qTs[hsl, 0, 0:512],
                            start=True, stop=True,
                        )
                        pS = ptp.tile([128, 512], BF, tag="pA")
                        nc.scalar.activation(pS, sS, AF.Exp, bias=zbias, scale=0.125)
                        nc.tensor.matmul(
                            o[0 : HD + 1, :],
                            lhsT=v_s[:, sb, half * 65 : half * 65 + 65],
                            rhs=pS,
                            start=(sb == 0), stop=(sb == NSB - 1),
                        )

                done_sb = 0
                for tb in range(NTB):
                    xt = xp.tile([128, NCC, 128], BF, tag="xt")
                    nc.sync.dma_start(
                        xt,
                        xT.ap().rearrange("(cc p) t -> p cc t", p=128)[
                            :, :, tb * 128 : (tb + 1) * 128
                        ],
                    )
                    if tb == 0:
                        nc.sync.dma_start(
                            wv_s, wvT.ap().rearrange("(cc p) j -> p cc j", p=128)
                        )
                        nc.sync.dma_start(
                            cos_s, cosn.ap().rearrange("(tc p) d -> p tc d", p=128)
                        )
                        nc.sync.dma_start(
                            sin_s, sinn.ap().rearrange("(tc p) d -> p tc d", p=128)
                        )
                        nc.sync.dma_start(wo_s, woS.ap())
                    proj_qk(tb, xt)
                    vproj(tb, xt)
                    emit_transposes(keep=2)
                    # stream the (hp0, tw0) attention for ready s-blocks
                    # (tb-5: tw0's q transposes land by tb=5 with the keep=2 lag)
                    while done_sb <= tb - 5:
                        attn_sb_a(done_sb)
                        done_sb += 1
                emit_transposes()
                while done_sb < NSB:
                    attn_sb_a(done_sb)
                    done_sb += 1
                norm_pair(0, 0, oA0, oB0)

            # ---- phase B: remaining 15 (hp, tw) attention pairs with
            # engine-routed exp; out-projection dripped ----
            with (
                tc.tile_pool(name="psS", bufs=2, space="PSUM") as psS,
                tc.tile_pool(name="psO", bufs=4, space="PSUM") as psO,
            ):
                GRP = 8
                outproj_pending = []

                def emit_outproj(n):
                    while outproj_pending:
                        if n <= 0:
                            return
                        n -= 1
                        tw_, cb = outproj_pending.pop(0)
                        cbsl = slice(cb * 128, (cb + 1) * 128)
                        osl = slice(tw_ * 512, (tw_ + 1) * 512)
                        po = psO.tile([128, 512], F32, tag="o")
                        for pair in range(2):
                            nc.tensor.matmul(
                                po, lhsT=wo_s[:, pair, cbsl], rhs=yP[:, pair, osl],
                                start=(pair == 0), stop=(pair == 1),
                            )
                        st = stagep.tile([128, 512], F32, tag="st")
                        nc.vector.tensor_copy(st, po)
                        nc.sync.dma_start(pout.ap()[cbsl, osl], st)

                pairs = [(1, 0)] + [
                    (hp, tw) for tw in range(1, NTW) for hp in range(2)
                ]
                for hp, tw in pairs:
                    oA = psO.tile([128, 512], F32, tag="o")
                    oB = psO.tile([128, 512], F32, tag="o")
                    for g in range(NSB // GRP):
                        pend = []
                        for i in range(GRP):
                            sb = g * GRP + i
                            sAB = emit_qk(hp, tw, sb, psS)
                            m = sb % 8
                            route = 1 if m in DVE_SB else (2 if m in GPS_SB else 0)
                            pend.append((sb, emit_exp(sAB, route)))
                        emit_outproj(2)
                        for sb, pAB in pend:
                            emit_pv(hp, sb, pAB, oA, oB)
                    norm_pair(hp, tw, oA, oB)
                    if hp == 1:
                        emit_outproj(8)  # drain leftovers
                        outproj_pending.extend((tw, cb) for cb in range(8))
                emit_outproj(8)  # final tw's out-projection

    nc.compile()
    return nc


def _get_program():
    global _PROGRAM
    if _PROGRAM is None:
        _PROGRAM = _build_program()
    return _PROGRAM


def make_in_maps(x, wq, wk, wv, wo):
    """Host-side sharding/layout prep: per-core input dicts."""
    x = np.asarray(x, dtype=np.float32)
    wq = np.asarray(wq, dtype=np.float32)
    wk = np.asarray(wk, dtype=np.float32)
    wv = np.asarray(wv, dtype=np.float32)
    wo = np.asarray(wo, dtype=np.float32)
    cos, sinS = _rope_tables_np()

    xT_b = [np.ascontiguousarray(x[b].T).astype(BF16) for b in range(B)]
    in_maps = []
    for c in range(N_CORES):
        b, hg = divmod(c, HPC)
        jsl = slice(hg * JC, (hg + 1) * JC)
        wqTc = np.ascontiguousarray(wq[jsl, :].T).astype(BF16)
        wkTc = np.ascontiguousarray(wk[jsl, :].T).astype(BF16)
        wvTc = np.ascontiguousarray(wv[jsl, :].T).astype(BF16)
        # woS[p, pair, co]: p<64 -> head 2*pair d=p ; p>=64 -> head 2*pair+1
        wo_cols = wo[:, jsl].reshape(DIM, HPC, HD)  # [co, h, d]
        woSc = np.zeros((128, 2, DIM), dtype=np.float32)
        for pair in range(2):
            woSc[0:64, pair, :] = wo_cols[:, 2 * pair, :].T
            woSc[64:128, pair, :] = wo_cols[:, 2 * pair + 1, :].T
        in_maps.append(
            {
                "xT": xT_b[b],
                "wqT": wqTc,
                "wkT": wkTc,
                "wvT": wvTc,
                "woS": woSc.astype(BF16),
                "cosn": cos,
                "sinn": sinS,
                "chain": _ZCHAIN,
            }
        )
    return in_maps


_ZCHAIN = np.zeros((DIM, T), dtype=np.float32)


def assemble(results):
    """Host-side unshard: sum 4 head-group partials per batch, transpose."""
    out = np.zeros((B, T, DIM), dtype=np.float32)
    for b in range(B):
        acc = np.zeros((DIM, T), dtype=np.float32)
        for hg in range(HPC):
            acc += results[b * HPC + hg]["pout"]
        out[b] = acc.T
    return out


def kernel(x, wq, wk, wv, wo):
    from concourse.bass_utils import run_bass_kernel_spmd

    nc = _get_program()
    in_maps = make_in_maps(x, wq, wk, wv, wo)
    res = run_bass_kernel_spmd(nc, in_maps, core_ids=list(range(N_CORES)))
    return assemble(res.results)


if __name__ == "__main__":
    nc = _get_program()
    print("program built + compiled OK")
